# revision 2
# baseline (speedup 1.0000x reference)
"""Depthwise causal Conv1d (K=16) for x:(4, 2048, 8192) f32 on 8 TRN2 NeuronCores.

Strategy (tensor-parallel over channels, no cross-core communication):
  - Each core owns 256 channels (2048 / 8) for all 4 batches.
  - Time axis is laid out in 128-sample blocks on the SBUF partition axis,
    so the depthwise conv of each channel becomes one banded-Toeplitz
    matmul on the TensorEngine, plus a tiny corner matmul for the
    block-boundary (previous block's last 15 samples):
        psum[m, (b,j)] = sum_p A_rev[p, m] * X[p, (b, j)]
                       + sum_p B_rev[p, m] * X[p, (b, j-1)]
    with the contraction index p time-REVERSED within each block (this makes
    every host/device access pattern a purely positive-stride AP):
        A_rev[p, m] = w[142 - p - m]  for 127 <= p + m <= 142, else 0
        B_rev[p, m] = w[14 - p - m]   for p + m <= 14, else 0
        X[p, (b, j)] = x[b, c, 128*(j-1) + (127 - p)]  (j = 0 is a zero col)
  - The band matrices (A: 64 KiB/channel, B: 7.5 KiB/channel) are built on
    the host and shipped in a p-major layout so every DMA descriptor is a
    large contiguous run.
  - Epilogue: PSUM -> SBUF copy with per-channel bias add on DVE/ACT, then
    one large store per 32-channel chunk.

The host does the sharding + block-layout transposes with numpy; the device
kernel sees only dense p-major arrays.
"""

import os
import sys

import numpy as np

if "/opt/trn_rl_repo" not in sys.path:
    sys.path.insert(0, "/opt/trn_rl_repo")

import concourse.bacc as bacc
import concourse.mybir as mybir
import concourse.tile as tile
from concourse.bass_utils import run_bass_kernel_spmd

F32 = mybir.dt.float32
F32R = mybir.dt.float32r

N_CORES = 8
B = 4            # batch
DIM = 2048       # channels
T = 8192         # time
K = 16           # conv taps
C = DIM // N_CORES   # channels per core = 256
P = 128          # time-block size (partition dim)
NBLK = T // P    # 64 blocks per (batch, channel)
NCJ = NBLK + 1   # x cols per batch incl. leading zero col = 65
XCOLS = B * NCJ  # 260
OCOLS = B * NBLK  # 256
CH = 32          # channels per device chunk
NCHUNK = C // CH  # 8

# matmul dtype: float32 (exact, 4 cyc/row) or float32r (fast, 1 cyc/row)
MM_DTYPE = F32R if os.environ.get("CONV_MM_F32R", "1") == "1" else F32

_compiled_nc = None


def _build_kernel():
    nc = bacc.Bacc(None)

    xin = nc.declare_dram_parameter("xin", [P, C, XCOLS], MM_DTYPE, isOutput=False)
    a_in = nc.declare_dram_parameter("a_in", [P, C, P], MM_DTYPE, isOutput=False)
    b_in = nc.declare_dram_parameter("b_in", [P, C, K - 1], MM_DTYPE, isOutput=False)
    bias_in = nc.declare_dram_parameter("bias_in", [P, C], F32, isOutput=False)
    yout = nc.declare_dram_parameter("yout", [P, C, OCOLS], F32, isOutput=True)

    with tile.TileContext(nc) as tc:
        with (
            tc.tile_pool(name="xpool", bufs=2) as xpool,
            tc.tile_pool(name="apool", bufs=2) as apool,
            tc.tile_pool(name="bpool", bufs=2) as bpool,
            tc.tile_pool(name="opool", bufs=2) as opool,
            tc.tile_pool(name="cpool", bufs=1) as cpool,
            tc.tile_pool(name="psum", bufs=8, space="PSUM") as pspool,
        ):
            bias_t = cpool.tile([P, C], F32)
            nc.gpsimd.dma_start(out=bias_t[:], in_=bias_in[:])

            for chunk in range(NCHUNK):
                c0 = chunk * CH
                x_t = xpool.tile([P, CH * XCOLS], MM_DTYPE)
                a_t = apool.tile([P, CH * P], MM_DTYPE)
                b_t = bpool.tile([P, CH * (K - 1)], MM_DTYPE)
                o_t = opool.tile([P, CH * OCOLS], F32)

                nc.gpsimd.dma_start(
                    out=x_t[:].rearrange("p (c j) -> p c j", c=CH),
                    in_=xin[:, c0 : c0 + CH, :],
                )
                nc.gpsimd.dma_start(
                    out=a_t[:].rearrange("p (c m) -> p c m", c=CH),
                    in_=a_in[:, c0 : c0 + CH, :],
                )
                nc.gpsimd.dma_start(
                    out=b_t[:].rearrange("p (c m) -> p c m", c=CH),
                    in_=b_in[:, c0 : c0 + CH, :],
                )

                xv = x_t[:].rearrange("p (c b j) -> p c b j", c=CH, b=B)

                for i in range(CH):
                    ps = pspool.tile([P, OCOLS], F32)
                    psv = ps[:].rearrange("m (b j) -> m b j", b=B)
                    lhs_a = a_t[:, i * P : (i + 1) * P]
                    lhs_b = b_t[:, i * (K - 1) : (i + 1) * (K - 1)]
                    rhs_a = xv[:, i, :, 1:]
                    rhs_b = xv[:, i, :, 0:NBLK]
                    nc.tensor.matmul(psv, lhs_a, rhs_a, start=True, stop=False)
                    nc.tensor.matmul(
                        psv[0 : K - 1], lhs_b, rhs_b, start=False, stop=True
                    )
                    # psum -> sbuf with bias add
                    nc.vector.tensor_scalar_add(
                        o_t[:, i * OCOLS : (i + 1) * OCOLS],
                        ps[:],
                        bias_t[:, c0 + i : c0 + i + 1],
                    )

                nc.sync.dma_start(
                    out=yout[:, c0 : c0 + CH, :],
                    in_=o_t[:].rearrange("p (c j) -> p c j", c=CH),
                )

    nc.compile()
    return nc


def _get_nc():
    global _compiled_nc
    if _compiled_nc is None:
        _compiled_nc = _build_kernel()
    return _compiled_nc


def _prep_core(x, weight, bias, core):
    """Build the per-core input map (numpy only)."""
    cs = slice(core * C, (core + 1) * C)
    xs = x[:, cs, :]                       # [B, C, T]
    w = weight[cs, 0, :]                   # [C, K]
    bs = bias[cs]                          # [C]

    # x: [B, C, T] -> [P, C, B, NCJ] with time reversed inside each block and
    # a leading zero column per batch.
    xr = xs.reshape(B, C, NBLK, P)[:, :, :, ::-1]   # [B, C, j, p(reversed)]
    xin = np.zeros((P, C, B, NCJ), dtype=np.float32)
    xin[:, :, :, 1:] = xr.transpose(3, 1, 0, 2)     # [p, c, b, j]
    xin = xin.reshape(P, C, XCOLS)

    # A_rev[p, m] = w[142 - p - m] for 127 <= p+m <= 142
    idx = np.arange(P)[:, None] + np.arange(P)[None, :]   # p + m
    amask = (idx >= 127) & (idx <= 142)
    aidx = np.clip(142 - idx, 0, K - 1)
    a_mat = np.where(amask[None], w[:, aidx], 0.0)        # [C, P, P]
    a_in = np.ascontiguousarray(a_mat.transpose(1, 0, 2)).astype(np.float32)

    # B_rev[p, m] = w[14 - p - m] for p+m <= 14
    idxb = np.arange(P)[:, None] + np.arange(K - 1)[None, :]
    bmask = idxb <= 14
    bidx = np.clip(14 - idxb, 0, K - 1)
    b_mat = np.where(bmask[None], w[:, bidx], 0.0)        # [C, P, K-1]
    b_in = np.ascontiguousarray(b_mat.transpose(1, 0, 2)).astype(np.float32)

    bias_in = np.ascontiguousarray(
        np.broadcast_to(bs[None, :], (P, C))
    ).astype(np.float32)

    return {"xin": xin, "a_in": a_in, "b_in": b_in, "bias_in": bias_in}


def run(x, weight, bias, trace=False):
    nc = _get_nc()
    in_maps = [_prep_core(x, weight, bias, core) for core in range(N_CORES)]
    res = run_bass_kernel_spmd(nc, in_maps, list(range(N_CORES)), trace=trace)

    y = np.empty((B, DIM, T), dtype=np.float32)
    for core in range(N_CORES):
        yp = res.results[core]["yout"]                 # [P(m), C, B*NBLK]
        yc = yp.reshape(P, C, B, NBLK).transpose(2, 1, 3, 0)  # [B, C, j, m]
        y[:, core * C : (core + 1) * C, :] = yc.reshape(B, C, T)
    return y, res


def kernel(x, weight, bias):
    y, _ = run(
        np.asarray(x, dtype=np.float32),
        np.asarray(weight, dtype=np.float32),
        np.asarray(bias, dtype=np.float32),
    )
    return y
